# revision 40
# baseline (speedup 1.0000x reference)
"""Trainium2 Bass kernel for the supervoxel erode/edge loss module.

The reference pads a [B,X,Y] grid (offset 4*sx rows / 4*sy cols), tiles it
into 8x8 patches, zeroes each patch's last row/col of the mask channel,
erodes along both patch axes and sums eroded*edge.  The erode
`a*b + (1-a)*a + (1-b)*a` equals `2a - a^2 = 1 - (1-a)^2` with
a = m(i)*m(i+1), so the whole module collapses to a global elementwise
expression on the unpadded grid (validated to f64 exactness):

    mt(x,y) = mask[b,x,y,idx] * [(x+4sx)%8 != 7] * [(y+4sy)%8 != 7]
    ax = mt(x,y)*mt(x+1,y); ay = mt(x,y)*mt(x,y+1)   (zero past image edge)
    total = sum (1-(1-ax)^2) * (1-(1-ay)^2) * edge
    out = loss_old + total / (B * ((X+8)//8) * ((Y+8)//8))

Host-side packing (layout/precision only -- all arithmetic on device):
  * only mask channel `idx` matters (4x traffic cut); mask rows with
    x%8 == 7-4sx / cols with y%8 == 7-4sy are provably dead, as are edge
    rows x%8 in {6-4sx,7-4sx} and cols y%8 in {6-4sy,7-4sy}.
  * f32 -> f16: inputs are U[0,1) and the loss is a mean of ~10M terms;
    measured end-to-end rel err ~5e-6 (gate is 2e-2).
  mask -> [Bc, 897, 897] f16: 7 live row/col classes per group of 8, plus
    one zero pad row and col (exact zero shift-ins at image edges).
  edge -> [Bc, 768, 768] f16 (sy==0): LIVE slots only -- 6 live row
    classes x 6 live col classes per group; no zero slots shipped at all.

Rewrite of the 47.8us baseline: every work tile is live-slot-only
[2,6,128,6] so there is no dead slot anywhere (no memsets, 14% less
Q/t1/p2 work and edge DMA), and the per-image pipeline is

    DVE : ay = v*vy ; ax = v*vx        (tensor_tensor 2x f16, per chunk)
    ACT : SQ = (1-a)^2                  (Square, per chunk piece)
    DVE : Q  = SQ-1                     (tensor_scalar 4x, aliased into P)
    DVE : t1 = Qy*e ; p2 = Qx*t1        (tensor_tensor 2x)
    sum : earlier images via ACT Copy accum_out (off the critical path);
          the LAST image splits its x-join 50/50 -- half A via the 4x
          TS (fused with Qy over the contiguous [0,N+H) span, emitted
          BEFORE t1 so the scheduler can't reorder past it) + 2x TT +
          ACT Copy accum, half B via one 1x STT (sub+mul+accum) emitted
          last; the ACT accum hides under the STT so the post-t1 tail
          is ~3.9us instead of a full-width 5us STT.

Measured rates (HW): TT 2x ~1.8 el/ns, TS 4x ~3.5 (but TS WITH accum_out
lowers to TENSOR_SCALAR_CACHE_REDUCE at 1x -- avoid), STT/TTR/reduce 1x
~0.9, ACT ~1.15.  Custom DVE ops all run 1x (no perf_en).  A fused
ay||ax product op is impossible: TensorTensor ISA patterns cap at 3 free
dims.  With this split DVE is ~29us busy with zero idle from the first
product to the last STT, ACT ~22us -- the kernel is elementwise-bound,
not DMA-bound (5.9MB rides one sync-ring at ~390GB/s in ~15us).

Each image's mask is ONE sbuf tile filled by row-ranged sub-DMAs (img0:
rows 0:3/3:5/5:8 -- products start at ~10.5us when 690KB lands, and the
3:5 piece fills the 1.2us DVE gap the 0:3 products leave while 3:8
streams; later images 0:4/4:8 so their products start ~2.5us early.
1-row cuts regress (1794B ring lines are overhead-bound) and so does a
leading 2-row chunk: extra descriptors near the head stretch the whole
stream and add completion-straggler exposure, the main run-to-run
variance.  All input DMAs ride the sync
HWDGE ring in exact consumption order (mask img0, mask img1, edges).
The [128, 2Bc] accumulator columns partition-reduce through an idle-PE
ones-matmul so the output DMA is one 16B line ([128,k]-shaped outputs
cost ~1us in scattered 8B ring packets).  Host sums the per-core column
partials (the mean needs no collective).

Sharding: data-parallel over batch, B/8 images per core on 8 cores.
Beware DVFS: runs sporadically execute with the whole chip ~20% slower
(ACT_TABLE_LOAD 1539ns vs 1283ns is the tell) -- compare traces only
after normalizing by that fixed-work instruction.

Progression on HW (exec_time, core 0, full-clock runs): 47.9us baseline
-> 49.0 (live-only tiles + TS-accum tail: TS-accum is secretly 1x) ->
46.2 (single STT tail + PE-reduce output + split mask DMAs) -> ~46.3
(50/50 split STT/ACT tail join) -> ~44.3-45.8 mean ~45.1 (img0 mask as
0:3/3:8) -> ~43.9-44.1 (img0 as 0:3/3:5/5:8, mid-stream gap fill).
"""

import os
import sys

sys.path.insert(0, "/opt/trn_rl_repo")

import numpy as np

from concourse import bacc, bass, mybir, tile
from concourse.ap import AP
from concourse.bass_utils import run_bass_kernel_spmd

F32 = mybir.dt.float32
DTYPES = {
    "float16": (mybir.dt.float16, np.float16),
    "bfloat16": (mybir.dt.bfloat16, None),  # np dtype resolved lazily
}
N_CORES = 8
SHIFTS = [(0, 0), (1, 0), (0, 1), (1, 1)]

NG = 128           # row groups per image (X=1024 / 8)
YG = 128           # col groups per image
MROW = 7 * YG + 1  # packed mask row length (896 live + zero pad col)
MIMG = 897 * MROW  # elements per packed mask image ((896+1 pad row) * 897)


def _np_dtype(name):
    if name == "float16":
        return np.float16
    import ml_dtypes

    return ml_dtypes.bfloat16


def _geom(idx):
    """Keep-classes and live/dead slots for mask_index idx."""
    sx, sy = SHIFTS[idx]
    xdrop = (7 - 4 * sx) % 8
    ydrop = (7 - 4 * sy) % 8
    KR = [c for c in range(8) if c != xdrop]          # mask row classes kept
    KC = [c for c in range(8) if c != ydrop]          # mask col classes kept
    dead_x = {(6 - 4 * sx) % 8, (7 - 4 * sx) % 8}     # term row classes dead
    dead_y = {(6 - 4 * sy) % 8, (7 - 4 * sy) % 8}
    live_s = [i for i, c in enumerate(KR) if c not in dead_x]  # 6 slots
    live_j = [i for i, c in enumerate(KC) if c not in dead_y]  # 6 slots
    # live mask cols contiguous (sy==0) -> 6-wide edge col groups; else
    # (sy==1) 7-wide groups with the dead-term column zeroed
    WJ = 6 if live_j == list(range(6)) else 7
    return KR, KC, live_s, live_j, WJ


def _si_runs(srows, si_lo, si_hi):
    """Contiguous mask-row runs [(si0, w)] within live-slot range [si_lo, si_hi)."""
    out = []
    for si in range(si_lo, si_hi):
        if out and srows[si] == srows[si - 1] + 1 and si == out[-1][0] + out[-1][1]:
            out[-1] = (out[-1][0], out[-1][1] + 1)
        else:
            out.append((si, 1))
    return out


def _build_program(Bc: int, idx: int, join: str = "hybrid", dt_name: str = "float16"):
    """Per-core program. Inputs: mask [Bc,897,897], edge [Bc,S*128,WJ*128] in
    dt_name. Output: out [128, Bc] f32 per-partition partial sums."""
    _, _, live_s, _, WJ = _geom(idx)
    DT = DTYPES[dt_name][0]
    S = len(live_s)       # live term-row slots per group (6)
    srows = live_s        # mask slot-row feeding term slot si
    erow = WJ * YG        # edge row length
    eimg = S * NG * erow
    N = S * YG * WJ       # live elems per partition per image (4608)
    SQF = mybir.ActivationFunctionType.Square
    CPF = mybir.ActivationFunctionType.Copy
    SUB = mybir.AluOpType.subtract
    ADD = mybir.AluOpType.add

    # mask sub-DMA row cuts: split so compute starts as rows land and a
    # single straggling ring engine only stalls half an image's rows.
    def cuts_for(b):
        lo = srows[0]
        if b == 0:
            return [lo, lo + 3, lo + 5, 8]
        return [lo, (lo + 8) // 2, 8]

    nc = bacc.Bacc("TRN2", target_bir_lowering=False, debug=False)
    mask_h = nc.dram_tensor("mask", [Bc, 897, MROW], DT, kind="ExternalInput")
    edge_h = nc.dram_tensor("edge", [Bc, S * NG, erow], DT, kind="ExternalInput")
    out_h = nc.dram_tensor("out", [1, 2 * Bc], F32, kind="ExternalOutput")

    with tile.TileContext(nc) as tc:
        with (
            tc.tile_pool(name="mt", bufs=1) as mt_pool,
            tc.tile_pool(name="et", bufs=1) as et_pool,
            tc.tile_pool(name="wk", bufs=1) as wk_pool,
            tc.tile_pool(name="psum", bufs=1, space="PSUM") as ps_pool,
            tc.tile_pool(name="const", bufs=1) as c_pool,
        ):
            acc = c_pool.tile([128, 2 * Bc], F32)
            ones_t = c_pool.tile([128, 1], F32)
            nc.vector.memset(acc[:], 0.0)
            nc.vector.memset(ones_t[:], 1.0)

            mcs, Ps, SQs, t1s, ets = [], [], [], [], []
            # ---- phase 1: mask DMAs + products + squares ----
            # Pieces are emitted in per-image stream order.
            prog = []
            for b in range(Bc):
                mc = mt_pool.tile([128, 8 * MROW], DT, tag=f"m{b}", bufs=1)
                P = wk_pool.tile([128, 2 * N], DT, tag=f"P{b}", bufs=1)
                SQ = wk_pool.tile([128, 2 * N], DT, tag=f"SQ{b}", bufs=1)
                mcs.append(mc); Ps.append(P); SQs.append(SQ)
                prog.append([cuts_for(b), 0, 0])  # cuts, ay_done, ax_done

            # NOTE: a fused ay||ax op (broadcast A, stride MROW-1 leading
            # dim on B) fails codegen -- TensorTensor ISA patterns allow
            # at most 3 free dims, and the fusion needs 4.
            def emit_piece(b, ci):
                mc = mcs[b]
                mv = mc[:].rearrange("p (r y) -> p r y", r=8)
                Pv = Ps[b][:].rearrange("p (h s g j) -> p h s g j", h=2, s=S, j=WJ)
                Sv = SQs[b][:].rearrange("p (h s g j) -> p h s g j", h=2, s=S, j=WJ)

                def view(r0, w, coff):
                    return (
                        mv[:, r0 : r0 + w, coff : coff + 7 * YG]
                        .rearrange("p s (g j) -> p s g j", j=7)[:, :, :, 0:WJ]
                    )

                def half_tt(h, si0, w):
                    r0 = srows[si0]
                    bop = view(r0 + (0 if h == 0 else 1), w, 1 - h)
                    nc.vector.tensor_mul(Pv[:, h, si0 : si0 + w], view(r0, w, 0), bop)
                    nc.scalar.activation(
                        Sv[:, h, si0 : si0 + w], Pv[:, h, si0 : si0 + w],
                        SQF, bias=1.0, scale=-1.0,
                    )

                cuts, ay_done, ax_done = prog[b]
                ra, rb = cuts[ci - 1], cuts[ci]
                nc.sync.dma_start(
                    mc[:, ra * MROW : rb * MROW],
                    AP(mask_h, b * MIMG + ra * MROW,
                       [[7 * MROW, 128], [1, (rb - ra) * MROW]]),
                )
                ay_hi = sum(1 for s in srows if s < rb)
                ax_hi = sum(1 for s in srows if s + 1 < rb)
                for (si0, w) in _si_runs(srows, ay_done, ay_hi):
                    half_tt(0, si0, w)
                for (si0, w) in _si_runs(srows, ax_done, ax_hi):
                    half_tt(1, si0, w)
                prog[b][1], prog[b][2] = ay_hi, ax_hi

            # stream order: strictly per-image.  Interleaving img1's first
            # chunk ahead of ANY img0 rows measures worse (the join chain
            # gates on img0's squares; even slotting m1a before just the
            # last img0 chunk lost ~0.5-1us A/B).
            pieces = [(b, ci) for b in range(Bc)
                      for ci in range(1, len(prog[b][0]))]
            qp = [0, 0]  # img0 Q prefix already emitted: [y_slots, x_slots]
            for b, ci in pieces:
                emit_piece(b, ci)
                if b == 0 and ci == len(prog[0][0]) - 1 and Bc >= 2:
                    # partial Q0 over slots whose squares are already done:
                    # fills the ~1us DVE wait for img1's first mask chunk
                    c2 = prog[0][0][-2]
                    qp[0] = sum(1 for s in srows if s < c2)
                    qp[1] = sum(1 for s in srows if s + 1 < c2)
                    blk = YG * WJ
                    nc.vector.tensor_scalar(
                        Ps[0][:][:, 0 : qp[0] * blk],
                        SQs[0][:][:, 0 : qp[0] * blk], 1.0, None, op0=SUB)
                    nc.vector.tensor_scalar(
                        Ps[0][:][:, N : N + qp[1] * blk],
                        SQs[0][:][:, N : N + qp[1] * blk], 1.0, None, op0=SUB)

            # ---- edge DMAs (after all mask traffic on the sync ring) ----
            for b in range(Bc):
                et = et_pool.tile([128, N], DT, tag=f"e{b}", bufs=1)
                ets.append(et)
                nc.sync.dma_start(
                    et[:],
                    AP(edge_h, b * eimg, [[N, 128], [1, N]]),
                )

            # ---- phase 2: joins, per image (y-chain first) ----
            for b in range(Bc):
                Ph = Ps[b][:].rearrange("p (h z) -> p h z", h=2)
                Sh = SQs[b][:].rearrange("p (h z) -> p h z", h=2)
                t1 = wk_pool.tile([128, N], DT, tag=f"t1_{b}", bufs=1)
                if b < Bc - 1 and join == "hybrid":
                    # Q (both halves, one 4x op) aliased into P (P is dead
                    # once SQ is computed); t1/p2 at 2x; accumulate on ACT
                    # (off the critical path while later images compute).
                    blk = YG * WJ
                    nc.vector.tensor_scalar(
                        Ps[b][:][:, qp[0] * blk : N],
                        SQs[b][:][:, qp[0] * blk : N], 1.0, None, op0=SUB)
                    nc.vector.tensor_scalar(
                        Ps[b][:][:, N + qp[1] * blk :],
                        SQs[b][:][:, N + qp[1] * blk :], 1.0, None, op0=SUB)
                    nc.vector.tensor_mul(t1[:], Ph[:, 0], ets[b][:])
                    # p2 aliased into SQ's y-half (dead once Q is computed)
                    nc.vector.tensor_mul(Sh[:, 0], Ph[:, 1], t1[:])
                    nc.scalar.activation(
                        Sh[:, 1], Sh[:, 0], CPF, accum_out=acc[:, b : b + 1]
                    )
                else:
                    # tail image: the x-side join splits 50/50 so the two
                    # accumulation engines overlap -- half A rides 4x TS (one
                    # op fused with Qy over the contiguous [0, N+H) span) +
                    # 2x TT + ACT Copy accum, half B rides a single 1x STT
                    # (subtract+multiply+accum) emitted last.  Qx-A runs
                    # BEFORE t1 so its completion is semaphore-visible when
                    # t1 ends and the scheduler cannot slip the STT ahead of
                    # p2-A (which would push the ACT accum onto the tail).
                    H = N // 2
                    nc.vector.tensor_scalar(
                        Ps[b][:][:, 0 : N + H], SQs[b][:][:, 0 : N + H],
                        1.0, None, op0=SUB,
                    )
                    nc.vector.tensor_mul(t1[:], Ph[:, 0], ets[b][:])
                    nc.vector.tensor_mul(
                        Sh[:, 0][:, 0:H], Ph[:, 1][:, 0:H], t1[:, 0:H]
                    )
                    nc.scalar.activation(
                        Sh[:, 1][:, 0:H], Sh[:, 0][:, 0:H], CPF,
                        accum_out=acc[:, Bc + b : Bc + b + 1],
                    )
                    nc.vector.scalar_tensor_tensor(
                        Sh[:, 0][:, H:], Sh[:, 1][:, H:], 1.0, t1[:, H:],
                        op0=SUB, op1=mybir.AluOpType.mult,
                        accum_out=acc[:, b : b + 1],
                    )

            # partition-reduce via an idle-PE ones-matmul so the output DMA
            # is one tiny line instead of 128 scattered 8B lines
            out_ps = ps_pool.tile([1, 2 * Bc], F32)
            nc.tensor.matmul(out_ps[:], ones_t[:], acc[:], start=True, stop=True)
            out_sb = c_pool.tile([1, 2 * Bc], F32)
            nc.vector.tensor_copy(out_sb[:], out_ps[:])
            nc.sync.dma_start(out_h.ap(), out_sb[:])

    nc.compile()
    return nc


def _pack_host(mask, edge, idx, dt_name="float16"):
    """Pack f32 [B,X,Y,{4,1}] inputs to the device layouts."""
    npdt = _np_dtype(dt_name)
    B, X, Y, _ = mask.shape
    KR, KC, live_s, live_j, WJ = _geom(idx)
    S = len(live_s)
    erow = WJ * YG
    m = mask[..., idx].reshape(B, NG, 8, Y)[:, :, KR, :]
    m = m.reshape(B, 7 * NG, YG, 8)[..., KC]
    mdev = np.zeros((B, 897, MROW), npdt)
    mdev[:, :896, :896] = m.reshape(B, 896, 896).astype(npdt)
    # edge: live slots only; row si of group g holds edge row class
    # KR[live_s[si]].  col slot jj maps to orig class KC[jj] (WJ==7) or
    # KC[live_j[jj]] (WJ==6); dead-term columns stay zero.
    e = edge[..., 0]
    EC = list(range(7)) if WJ == 7 else live_j
    dead_cols = [jj for jj in range(WJ) if EC[jj] not in live_j]
    edev = np.zeros((B, NG, S, YG, WJ), npdt)
    for si, s in enumerate(live_s):
        c = KR[s]
        es = e.reshape(B, NG, 8, Y)[:, :, c, :].reshape(B, NG, YG, 8)
        es = es[..., [KC[jj] for jj in EC]].astype(npdt)
        if dead_cols:
            es[..., dead_cols] = 0
        edev[:, :, si] = es
    return mdev, np.ascontiguousarray(edev.reshape(B, S * NG, erow))


def _run(mask, edge, loss_old, idx, trace=False, niter=1, join="hybrid",
         dt_name=None):
    if dt_name is None:
        dt_name = os.environ.get("KDT", "float16")
    B, X, Y, _ = mask.shape
    assert B % N_CORES == 0
    Bc = B // N_CORES

    nc = _build_program(Bc, idx, join=join, dt_name=dt_name)
    mdev, edev = _pack_host(mask, edge, idx, dt_name)
    in_maps = [
        {
            "mask": mdev[i * Bc : (i + 1) * Bc],
            "edge": edev[i * Bc : (i + 1) * Bc],
        }
        for i in range(N_CORES)
    ]
    res = run_bass_kernel_spmd(nc, in_maps, list(range(N_CORES)), trace=trace)
    total = float(
        sum(np.asarray(res.results[i]["out"], np.float64).sum() for i in range(N_CORES))
    )
    n_patch = ((X + 8) // 8) * ((Y + 8) // 8)
    out = np.float32(np.asarray(loss_old, dtype=np.float32) + total / (B * n_patch))
    return np.asarray(out, dtype=np.float32), res


def kernel(resized_image, mask_combined, edge_map, loss_old, mask_index):
    mask = np.ascontiguousarray(np.asarray(mask_combined, dtype=np.float32))
    edge = np.ascontiguousarray(np.asarray(edge_map, dtype=np.float32))
    idx = int(np.asarray(mask_index))
    out, _ = _run(mask, edge, loss_old, idx)
    return out
